# revision 15
# baseline (speedup 1.0000x reference)
"""DCN CrossNetwork kernel for Trainium2 (8 NeuronCores, data-parallel).

Reference computation (B=16384, D=1024, L=4 layers):
    x0 = x
    for c in range(L):
        s = x_c @ w_c               # (B,) row-wise dot
        x_{c+1} = x0 * s[:,None] + b_c + x_c

Algebra: every iterate has the form  x_c = x0 * a_c + r_c  with a per-row
scalar a_c and a row-independent vector r_c = sum_{j<c} b_j.  Then
    s_c   = a_c * (x0 . w_c) + r_c . w_c
    a_{c+1} = a_c * (1 + U_c) + d_c,   U_c = x0 . w_c,  d_c = r_c . w_c
    out   = x0 * a_L + r_L
So the device kernel only needs U = x0 @ W^T (TensorE), a 4-step scan
(VectorE tensor_tensor_scan), and one fused (x0 * a + r4) op per tile
(scalar_tensor_tensor).  d_c / r_L are tiny host-precomputed constants
(O(L*D) work on the L x D parameters only).

The kernel is HBM-bound, so device I/O is 16-bit: the host casts x to
fp16 (rel err ~3e-4) and the device stores out as bf16 (out magnitude
reaches ~4e7, which overflows fp16; bf16 rounding is ~2e-3 rel err).
The host upcasts the result to f32.  Total rel err ~2.5e-3, well under
the 2e-2 gate, for half the HBM traffic of the f32 kernel.

Engine assignment (each engine has one job so no queue head-of-line
blocking): SP ring = x loads; ACT = PSUM->SBUF transpose copies (+ the
tiny u1 bias) and the const loads on its HWDGE ring; DVE = scan + final
fused op; Pool (SWDGE) = output stores; PE = transposes + U matmuls.

Sharding: batch dim split across 8 cores (2048 rows each); weights/biases
replicated.  No collectives.
"""

import sys

for _p in ("/opt/trn_rl_repo",):
    if _p not in sys.path:
        sys.path.insert(0, _p)

import numpy as np

B, D, L = 16384, 1024, 4
N_CORES = 8
B_SHARD = B // N_CORES       # 2048 rows per core
P = 128                      # SBUF partitions
N_TILES = B_SHARD // P       # 16 row-tiles per core
N_CHUNKS = D // P            # 8 column-chunks of 128

_BUILT = None  # cached (nc) bass program


DEFAULT_CFG = dict(
    dma_batch=1,      # b-tiles per load DMA (256KB at fp16 with 1)
    copy_eng="dve",   # xT PSUM->SBUF copy engine: "act" or "dve" (2x mode)
    x_bufs=6,
    xt_bufs=5,
    o_bufs=6,
    tp_bufs=3,
    up_bufs=2,
    sw_pipe=1,        # tiles of lag between transpose stage and U/final stage
    store_eng="scalar",  # "sync" | "scalar" | "gpsimd"
    store_sub=True,   # store each [128,D] tile as soon as its final op is done
    const_eng="sync",  # ring for the tiny const loads
    final_op="ts_tt",  # "stt" (1 DVE op) | "ts_tt" (DVE ts then tt add)
    final_tt_eng="gpsimd",  # engine for the tt add when final_op="ts_tt"
    u1_eng="act",     # "act" | "dve": engine for the +1 bias on U
)


def build_bass(iters=1, **cfg_over):
    """Build the per-core Bass/Tile program (SPMD: same program, 8 cores).

    iters > 1 unrolls the whole body multiple times (same data) — used only
    for steady-state benchmarking via the loop-delta method.
    """
    import concourse.bass as bass
    import concourse.bacc as bacc
    import concourse.mybir as mybir
    import concourse.tile as tile

    cfg = {**DEFAULT_CFG, **cfg_over}
    f32 = mybir.dt.float32
    f16 = mybir.dt.float16
    bf16 = mybir.dt.bfloat16
    Alu = mybir.AluOpType
    Act = mybir.ActivationFunctionType

    # Bacc (not raw Bass): its compile() legalizes multi-sem-wait
    # instructions that this container's walrus codegen rejects.
    nc = bacc.Bacc("TRN2", debug=False)

    x_d = nc.dram_tensor("x", [B_SHARD, D], f16, kind="ExternalInput").ap()
    # wt[p, 4c+i] = W[i, 128c+p]  (W^T packed per 128-chunk)
    wt_d = nc.dram_tensor("wt", [P, L * N_CHUNKS], f16, kind="ExternalInput").ap()
    # r4 replicated across partitions
    r4_d = nc.dram_tensor("r4", [P, D], bf16, kind="ExternalInput").ap()
    # d1[p, c] = d_c (replicated across partitions)
    d1_d = nc.dram_tensor("d1", [P, L], f32, kind="ExternalInput").ap()
    id_d = nc.dram_tensor("ident", [P, P], f16, kind="ExternalInput").ap()
    out_d = nc.dram_tensor("out", [B_SHARD, D], bf16, kind="ExternalOutput").ap()

    NB = cfg["dma_batch"]
    assert N_TILES % NB == 0

    def view(dram, r0, nb=None):
        # partition p holds rows r0 + p*nb .. r0+p*nb+nb-1 (8KB descriptors)
        nb = nb or NB
        return dram[r0 : r0 + nb * P, :].rearrange("(p nb) d -> p nb d", nb=nb)

    cengine = {"scalar": nc.scalar, "sync": nc.sync, "gpsimd": nc.gpsimd}[
        cfg["const_eng"]
    ]
    sengine = {"scalar": nc.scalar, "sync": nc.sync, "gpsimd": nc.gpsimd}[
        cfg["store_eng"]
    ]

    with tile.TileContext(nc) as tc:
        from contextlib import ExitStack

        with ExitStack() as ctx:
            cpool = ctx.enter_context(tc.tile_pool(name="consts", bufs=1))
            xpool = ctx.enter_context(tc.tile_pool(name="x", bufs=cfg["x_bufs"]))
            xtpool = ctx.enter_context(tc.tile_pool(name="xt", bufs=cfg["xt_bufs"]))
            opool = ctx.enter_context(tc.tile_pool(name="o", bufs=cfg["o_bufs"]))
            upool = ctx.enter_context(tc.tile_pool(name="u", bufs=3))
            apool = ctx.enter_context(tc.tile_pool(name="a", bufs=3))
            tmppool = ctx.enter_context(tc.tile_pool(name="tmp", bufs=3))
            tpsum = ctx.enter_context(
                tc.tile_pool(name="tp", bufs=cfg["tp_bufs"], space=bass.MemorySpace.PSUM)
            )
            upsum = ctx.enter_context(
                tc.tile_pool(name="up", bufs=cfg["up_bufs"], space=bass.MemorySpace.PSUM)
            )

            # ident gates the very first transpose: load it first on the same
            # ring as the x loads so it wins the DMA device early.  The other
            # consts are needed later in the first tile's chain, so they load
            # after the first x tile (see grp0_consts below).
            id_t = cpool.tile([P, P], f16)
            cengine.dma_start(id_t[:], id_d[:])
            wt_t = cpool.tile([P, L * N_CHUNKS], f16)
            r4_t = cpool.tile([P, D], bf16)
            d1_t = cpool.tile([P, L], f32)

            # Software-pipelined emission: stage A (load/transpose/copy) runs
            # `sw_pipe` tiles ahead of stage B (U-matmul/scan/final/store) so
            # the PE never stalls on the PSUM->SBUF copy between its
            # transpose burst and U-matmul burst for the same tile.
            pend = []
            grp0_consts = [True]

            def emit_B(rec):
                xt_t, x_s, o_t, o_slice, sub_ap, grp = rec
                up = upsum.tile([P, L], f32)
                for c in range(N_CHUNKS):
                    nc.tensor.matmul(
                        up[:],
                        xt_t[:, c * P : (c + 1) * P],   # lhsT [K=d, M=b]
                        wt_t[:, L * c : L * (c + 1)],   # rhs  [K=d, N=4]
                        start=(c == 0),
                        stop=(c == N_CHUNKS - 1),
                    )
                # u1 = 1 + U  (PSUM -> SBUF with bias)
                u1 = upool.tile([P, L], f32)
                if cfg["u1_eng"] == "dve":
                    nc.vector.tensor_scalar(u1[:], up[:], 1.0, None, op0=Alu.add)
                else:
                    nc.scalar.activation(u1[:], up[:], Act.Copy, bias=1.0)
                # scan: a_{c+1} = u1_c * a_c + d_c  -> a[:, 3] = a_4
                a_t = apool.tile([P, L], f32)
                nc.vector.tensor_tensor_scan(
                    a_t[:], u1[:], d1_t[:], initial=1.0,
                    op0=Alu.mult, op1=Alu.add,
                )
                a4 = a_t[:, L - 1 : L]
                if cfg["final_op"] == "ts_tt":
                    # tmp = x0 * a4 (DVE tensor_scalar, 4x mode), then
                    # out = tmp + r4 (tensor_tensor, 2x mode)
                    tmp = tmppool.tile([P, D], bf16)
                    nc.vector.tensor_scalar(tmp[:], x_s, a4, None, op0=Alu.mult)
                    tt_eng = nc.gpsimd if cfg["final_tt_eng"] == "gpsimd" else nc.vector
                    tt_eng.tensor_tensor(o_slice, tmp[:], r4_t[:], op=Alu.add)
                else:
                    # out = x0 * a4 + r4  (single fused op on DVE)
                    nc.vector.scalar_tensor_tensor(
                        o_slice, x_s, a4, r4_t[:],
                        op0=Alu.mult, op1=Alu.add,
                    )
                if cfg["store_sub"]:
                    sengine.dma_start(sub_ap, o_slice)
                else:
                    grp["done"] += 1
                    if grp["done"] == NB:
                        sengine.dma_start(grp["store_ap"], o_t[:])

            for g in range((N_TILES // NB) * iters):
                g = g % (N_TILES // NB)
                r0 = g * NB * P
                # batched load: [NB*128, D] rows -> SBUF [128, NB, D]
                x_t = xpool.tile([P, NB, D], f16)
                cs = cfg.get("col_split", 1)
                if cs > 1:
                    w = D // cs
                    for h in range(cs):
                        nc.sync.dma_start(
                            x_t[:, :, h * w : (h + 1) * w],
                            view(x_d, r0)[:, :, h * w : (h + 1) * w],
                        )
                else:
                    nc.sync.dma_start(x_t[:], view(x_d, r0))
                if g == 0 and grp0_consts[0]:
                    # wt/r4/d1 are first needed partway into the first tile's
                    # chain; load them after the first x tile so they don't
                    # delay it.
                    cengine.dma_start(wt_t[:], wt_d[:])
                    cengine.dma_start(r4_t[:], r4_d[:])
                    cengine.dma_start(d1_t[:], d1_d[:])
                    grp0_consts[0] = False
                o_t = opool.tile([P, NB, D], bf16)
                grp = {"done": 0, "store_ap": view(out_d, r0)}

                for s in range(NB):
                    x_s = x_t[:, s, :]
                    # store AP for just this subtile: partition p row r0+p*NB+s
                    sub_ap = view(out_d, r0)[:, s, :]
                    # --- transpose x tile chunk-wise via PE: xt[d, b] ---
                    # fp16 transpose must write an fp16 PSUM tile (2KB/part
                    # = one bank).
                    xt_t = xtpool.tile([P, D], f16)
                    tp = tpsum.tile([P, D], f16)
                    for c in range(N_CHUNKS):
                        nc.tensor.transpose(
                            tp[:, c * P : (c + 1) * P],
                            x_s[:, c * P : (c + 1) * P],
                            id_t[:],
                        )
                    if cfg["copy_eng"] == "dve":
                        nc.vector.tensor_copy(xt_t[:], tp[:])
                    else:
                        nc.scalar.copy(xt_t[:], tp[:])
                    pend.append((xt_t, x_s, o_t, o_t[:, s, :], sub_ap, grp))
                    while len(pend) > cfg["sw_pipe"]:
                        emit_B(pend.pop(0))

            while pend:
                emit_B(pend.pop(0))

    nc.compile()
    return nc


def host_constants(weights, biases):
    """Pack W^T and precompute d_c / r4 (tiny O(L*D) host work)."""
    import concourse.mybir as mybir

    np_bf16 = mybir.dt.np(mybir.dt.bfloat16)
    w = np.ascontiguousarray(np.asarray(weights, dtype=np.float32))
    b = np.ascontiguousarray(np.asarray(biases, dtype=np.float32))
    r = np.zeros(D, np.float32)
    d_vec = np.zeros(L, np.float32)
    for c in range(L):
        d_vec[c] = np.float32(r @ w[c])
        r = r + b[c]
    # wt[p, 4c+i] = W[i, 128c+p]
    wt = np.transpose(w.reshape(L, N_CHUNKS, P), (2, 1, 0)).reshape(P, N_CHUNKS * L)
    wt = np.ascontiguousarray(wt.astype(np.float16))
    r4_rep = np.ascontiguousarray(np.broadcast_to(r, (P, D)).astype(np_bf16))
    d1_rep = np.ascontiguousarray(np.broadcast_to(d_vec, (P, L)))
    ident = np.eye(P, dtype=np.float16)
    return wt, r4_rep, d1_rep, ident


def device_inputs(x, weights, biases):
    """Full f32 inputs -> per-core device input maps (16-bit x)."""
    x16 = np.asarray(x, dtype=np.float16)
    wt, r4_rep, d1_rep, ident = host_constants(weights, biases)
    return [
        {
            "x": np.ascontiguousarray(x16[c * B_SHARD : (c + 1) * B_SHARD]),
            "wt": wt,
            "r4": r4_rep,
            "d1": d1_rep,
            "ident": ident,
        }
        for c in range(N_CORES)
    ]


def _get_built():
    global _BUILT
    if _BUILT is None:
        _BUILT = build_bass()
    return _BUILT


def kernel(x, weights, biases, _trace=False):
    from concourse.bass_utils import run_bass_kernel_spmd

    x = np.asarray(x, dtype=np.float32)
    assert x.shape == (B, D), x.shape
    in_maps = device_inputs(x, weights, biases)

    nc = _get_built()
    res = run_bass_kernel_spmd(nc, in_maps, list(range(N_CORES)), trace=_trace)
    out = np.concatenate(
        [np.asarray(res.results[c]["out"]).astype(np.float32) for c in range(N_CORES)],
        axis=0,
    )
    if _trace:
        kernel.last_results = res
    return out


# revision 16
# speedup vs baseline: 1.0574x; 1.0574x over previous
"""DCN CrossNetwork kernel for Trainium2 (8 NeuronCores, data-parallel).

Reference computation (B=16384, D=1024, L=4 layers):
    x0 = x
    for c in range(L):
        s = x_c @ w_c               # (B,) row-wise dot
        x_{c+1} = x0 * s[:,None] + b_c + x_c

Algebra: every iterate has the form  x_c = x0 * a_c + r_c  with a per-row
scalar a_c and a row-independent vector r_c = sum_{j<c} b_j.  Then
    s_c   = a_c * (x0 . w_c) + r_c . w_c
    a_{c+1} = a_c * (1 + U_c) + d_c,   U_c = x0 . w_c,  d_c = r_c . w_c
    out   = x0 * a_L + r_L
So the device kernel only needs U = x0 @ W^T (TensorE), a 4-step scan
(VectorE tensor_tensor_scan), and one fused (x0 * a + r4) op per tile
(scalar_tensor_tensor).  d_c / r_L are tiny host-precomputed constants
(O(L*D) work on the L x D parameters only).

The kernel is HBM-bound, so device I/O is 16-bit: the host casts x to
fp16 (rel err ~3e-4) and the device stores out as bf16 (out magnitude
reaches ~4e7, which overflows fp16; bf16 rounding is ~2e-3 rel err).
The host upcasts the result to f32.  Total rel err ~2.5e-3, well under
the 2e-2 gate, for half the HBM traffic of the f32 kernel.

Engine assignment (each engine has one job so no queue head-of-line
blocking): SP ring = x loads; ACT = PSUM->SBUF transpose copies (+ the
tiny u1 bias) and the const loads on its HWDGE ring; DVE = scan + final
fused op; Pool (SWDGE) = output stores; PE = transposes + U matmuls.

Sharding: batch dim split across 8 cores (2048 rows each); weights/biases
replicated.  No collectives.
"""

import sys

for _p in ("/opt/trn_rl_repo",):
    if _p not in sys.path:
        sys.path.insert(0, _p)

import numpy as np

B, D, L = 16384, 1024, 4
N_CORES = 8
B_SHARD = B // N_CORES       # 2048 rows per core
P = 128                      # SBUF partitions
N_TILES = B_SHARD // P       # 16 row-tiles per core
N_CHUNKS = D // P            # 8 column-chunks of 128

_BUILT = None  # cached (nc) bass program


DEFAULT_CFG = dict(
    dma_batch=1,      # b-tiles per load DMA (256KB at fp16 with 1)
    copy_eng="act",   # xT PSUM->SBUF copy engine: "act" or "dve"
    x_bufs=6,
    xt_bufs=5,
    o_bufs=6,
    tp_bufs=3,
    up_bufs=2,
    sw_pipe=1,        # tiles of lag between transpose stage and U/final stage
    store_eng="gpsimd",  # "sync" | "scalar" | "gpsimd"
    store_sub=True,   # store each [128,D] tile as soon as its final op is done
    const_eng="sync",  # ring for the tiny const loads
    final_op="ts_tt",  # "stt" (1 DVE op) | "ts_tt" (DVE ts then tt add)
    final_tt_eng="dve",  # engine for the tt add when final_op="ts_tt"
    u1_eng="dve",     # "act" | "dve": engine for the +1 bias on U
)


def build_bass(iters=1, **cfg_over):
    """Build the per-core Bass/Tile program (SPMD: same program, 8 cores).

    iters > 1 unrolls the whole body multiple times (same data) — used only
    for steady-state benchmarking via the loop-delta method.
    """
    import concourse.bass as bass
    import concourse.bacc as bacc
    import concourse.mybir as mybir
    import concourse.tile as tile

    cfg = {**DEFAULT_CFG, **cfg_over}
    f32 = mybir.dt.float32
    f16 = mybir.dt.float16
    bf16 = mybir.dt.bfloat16
    Alu = mybir.AluOpType
    Act = mybir.ActivationFunctionType

    # Bacc (not raw Bass): its compile() legalizes multi-sem-wait
    # instructions that this container's walrus codegen rejects.
    nc = bacc.Bacc("TRN2", debug=False)

    x_d = nc.dram_tensor("x", [B_SHARD, D], f16, kind="ExternalInput").ap()
    # wt[p, 4c+i] = W[i, 128c+p]  (W^T packed per 128-chunk)
    wt_d = nc.dram_tensor("wt", [P, L * N_CHUNKS], f16, kind="ExternalInput").ap()
    # r4 replicated across partitions
    r4_d = nc.dram_tensor("r4", [P, D], bf16, kind="ExternalInput").ap()
    # d1[p, c] = d_c (replicated across partitions)
    d1_d = nc.dram_tensor("d1", [P, L], f32, kind="ExternalInput").ap()
    id_d = nc.dram_tensor("ident", [P, P], f16, kind="ExternalInput").ap()
    out_d = nc.dram_tensor("out", [B_SHARD, D], bf16, kind="ExternalOutput").ap()

    NB = cfg["dma_batch"]
    assert N_TILES % NB == 0

    def view(dram, r0, nb=None):
        # partition p holds rows r0 + p*nb .. r0+p*nb+nb-1 (8KB descriptors)
        nb = nb or NB
        return dram[r0 : r0 + nb * P, :].rearrange("(p nb) d -> p nb d", nb=nb)

    cengine = {"scalar": nc.scalar, "sync": nc.sync, "gpsimd": nc.gpsimd}[
        cfg["const_eng"]
    ]
    sengine = {"scalar": nc.scalar, "sync": nc.sync, "gpsimd": nc.gpsimd}[
        cfg["store_eng"]
    ]

    with tile.TileContext(nc) as tc:
        from contextlib import ExitStack

        with ExitStack() as ctx:
            cpool = ctx.enter_context(tc.tile_pool(name="consts", bufs=1))
            xpool = ctx.enter_context(tc.tile_pool(name="x", bufs=cfg["x_bufs"]))
            xtpool = ctx.enter_context(tc.tile_pool(name="xt", bufs=cfg["xt_bufs"]))
            opool = ctx.enter_context(tc.tile_pool(name="o", bufs=cfg["o_bufs"]))
            upool = ctx.enter_context(tc.tile_pool(name="u", bufs=3))
            apool = ctx.enter_context(tc.tile_pool(name="a", bufs=3))
            tmppool = ctx.enter_context(tc.tile_pool(name="tmp", bufs=3))
            tpsum = ctx.enter_context(
                tc.tile_pool(name="tp", bufs=cfg["tp_bufs"], space=bass.MemorySpace.PSUM)
            )
            upsum = ctx.enter_context(
                tc.tile_pool(name="up", bufs=cfg["up_bufs"], space=bass.MemorySpace.PSUM)
            )

            # ident gates the very first transpose: load it first on the same
            # ring as the x loads so it wins the DMA device early.  The other
            # consts are needed later in the first tile's chain, so they load
            # after the first x tile (see grp0_consts below).
            id_t = cpool.tile([P, P], f16)
            cengine.dma_start(id_t[:], id_d[:])
            wt_t = cpool.tile([P, L * N_CHUNKS], f16)
            r4_t = cpool.tile([P, D], bf16)
            d1_t = cpool.tile([P, L], f32)

            # Software-pipelined emission: stage A (load/transpose/copy) runs
            # `sw_pipe` tiles ahead of stage B (U-matmul/scan/final/store) so
            # the PE never stalls on the PSUM->SBUF copy between its
            # transpose burst and U-matmul burst for the same tile.
            pend = []
            grp0_consts = [True]

            def emit_B(rec):
                xt_t, x_s, o_t, o_slice, sub_ap, grp = rec
                up = upsum.tile([P, L], f32)
                for c in range(N_CHUNKS):
                    nc.tensor.matmul(
                        up[:],
                        xt_t[:, c * P : (c + 1) * P],   # lhsT [K=d, M=b]
                        wt_t[:, L * c : L * (c + 1)],   # rhs  [K=d, N=4]
                        start=(c == 0),
                        stop=(c == N_CHUNKS - 1),
                    )
                # u1 = 1 + U  (PSUM -> SBUF with bias)
                u1 = upool.tile([P, L], f32)
                if cfg["u1_eng"] == "dve":
                    nc.vector.tensor_scalar(u1[:], up[:], 1.0, None, op0=Alu.add)
                else:
                    nc.scalar.activation(u1[:], up[:], Act.Copy, bias=1.0)
                # scan: a_{c+1} = u1_c * a_c + d_c  -> a[:, 3] = a_4
                a_t = apool.tile([P, L], f32)
                nc.vector.tensor_tensor_scan(
                    a_t[:], u1[:], d1_t[:], initial=1.0,
                    op0=Alu.mult, op1=Alu.add,
                )
                a4 = a_t[:, L - 1 : L]
                if cfg["final_op"] == "ts_tt":
                    # tmp = x0 * a4 (DVE tensor_scalar, 4x mode), then
                    # out = tmp + r4 (tensor_tensor, 2x mode)
                    tmp = tmppool.tile([P, D], bf16)
                    nc.vector.tensor_scalar(tmp[:], x_s, a4, None, op0=Alu.mult)
                    tt_eng = nc.gpsimd if cfg["final_tt_eng"] == "gpsimd" else nc.vector
                    tt_eng.tensor_tensor(o_slice, tmp[:], r4_t[:], op=Alu.add)
                else:
                    # out = x0 * a4 + r4  (single fused op on DVE)
                    nc.vector.scalar_tensor_tensor(
                        o_slice, x_s, a4, r4_t[:],
                        op0=Alu.mult, op1=Alu.add,
                    )
                if cfg["store_sub"]:
                    sengine.dma_start(sub_ap, o_slice)
                else:
                    grp["done"] += 1
                    if grp["done"] == NB:
                        sengine.dma_start(grp["store_ap"], o_t[:])

            for g in range((N_TILES // NB) * iters):
                g = g % (N_TILES // NB)
                r0 = g * NB * P
                # batched load: [NB*128, D] rows -> SBUF [128, NB, D]
                x_t = xpool.tile([P, NB, D], f16)
                cs = cfg.get("col_split", 1)
                if cs > 1:
                    w = D // cs
                    for h in range(cs):
                        nc.sync.dma_start(
                            x_t[:, :, h * w : (h + 1) * w],
                            view(x_d, r0)[:, :, h * w : (h + 1) * w],
                        )
                else:
                    nc.sync.dma_start(x_t[:], view(x_d, r0))
                if g == 0 and grp0_consts[0]:
                    # wt/r4/d1 are first needed partway into the first tile's
                    # chain; load them after the first x tile so they don't
                    # delay it.
                    cengine.dma_start(wt_t[:], wt_d[:])
                    cengine.dma_start(r4_t[:], r4_d[:])
                    cengine.dma_start(d1_t[:], d1_d[:])
                    grp0_consts[0] = False
                o_t = opool.tile([P, NB, D], bf16)
                grp = {"done": 0, "store_ap": view(out_d, r0)}

                for s in range(NB):
                    x_s = x_t[:, s, :]
                    # store AP for just this subtile: partition p row r0+p*NB+s
                    sub_ap = view(out_d, r0)[:, s, :]
                    # --- transpose x tile chunk-wise via PE: xt[d, b] ---
                    # fp16 transpose must write an fp16 PSUM tile (2KB/part
                    # = one bank).
                    xt_t = xtpool.tile([P, D], f16)
                    tp = tpsum.tile([P, D], f16)
                    for c in range(N_CHUNKS):
                        nc.tensor.transpose(
                            tp[:, c * P : (c + 1) * P],
                            x_s[:, c * P : (c + 1) * P],
                            id_t[:],
                        )
                    if cfg["copy_eng"] == "dve":
                        nc.vector.tensor_copy(xt_t[:], tp[:])
                    else:
                        nc.scalar.copy(xt_t[:], tp[:])
                    pend.append((xt_t, x_s, o_t, o_t[:, s, :], sub_ap, grp))
                    while len(pend) > cfg["sw_pipe"]:
                        emit_B(pend.pop(0))

            while pend:
                emit_B(pend.pop(0))

    nc.compile()
    return nc


def host_constants(weights, biases):
    """Pack W^T and precompute d_c / r4 (tiny O(L*D) host work)."""
    import concourse.mybir as mybir

    np_bf16 = mybir.dt.np(mybir.dt.bfloat16)
    w = np.ascontiguousarray(np.asarray(weights, dtype=np.float32))
    b = np.ascontiguousarray(np.asarray(biases, dtype=np.float32))
    r = np.zeros(D, np.float32)
    d_vec = np.zeros(L, np.float32)
    for c in range(L):
        d_vec[c] = np.float32(r @ w[c])
        r = r + b[c]
    # wt[p, 4c+i] = W[i, 128c+p]
    wt = np.transpose(w.reshape(L, N_CHUNKS, P), (2, 1, 0)).reshape(P, N_CHUNKS * L)
    wt = np.ascontiguousarray(wt.astype(np.float16))
    r4_rep = np.ascontiguousarray(np.broadcast_to(r, (P, D)).astype(np_bf16))
    d1_rep = np.ascontiguousarray(np.broadcast_to(d_vec, (P, L)))
    ident = np.eye(P, dtype=np.float16)
    return wt, r4_rep, d1_rep, ident


def device_inputs(x, weights, biases):
    """Full f32 inputs -> per-core device input maps (16-bit x)."""
    x16 = np.asarray(x, dtype=np.float16)
    wt, r4_rep, d1_rep, ident = host_constants(weights, biases)
    return [
        {
            "x": np.ascontiguousarray(x16[c * B_SHARD : (c + 1) * B_SHARD]),
            "wt": wt,
            "r4": r4_rep,
            "d1": d1_rep,
            "ident": ident,
        }
        for c in range(N_CORES)
    ]


def _get_built():
    global _BUILT
    if _BUILT is None:
        _BUILT = build_bass()
    return _BUILT


def kernel(x, weights, biases, _trace=False):
    from concourse.bass_utils import run_bass_kernel_spmd

    x = np.asarray(x, dtype=np.float32)
    assert x.shape == (B, D), x.shape
    in_maps = device_inputs(x, weights, biases)

    nc = _get_built()
    res = run_bass_kernel_spmd(nc, in_maps, list(range(N_CORES)), trace=_trace)
    out = np.concatenate(
        [np.asarray(res.results[c]["out"]).astype(np.float32) for c in range(N_CORES)],
        axis=0,
    )
    if _trace:
        kernel.last_results = res
    return out
